# revision 13
# baseline (speedup 1.0000x reference)
"""Trainium2 Bass kernel for PointFeatureConv (KNN + edge MLP + mean + out MLP).

Self-contained: builds a Bass/Tile program, shards queries across 8 NeuronCores,
runs via run_bass_kernel_spmd, reassembles the full output.

Algorithm per core (data-parallel over queries; points replicated):
  Phase K (KNN): for each 128-query block, PE computes scores
      s[i,j] = 2*q_i.p_j - |p_j|^2   (argmax_j s = nearest neighbor)
    into PSUM in SW-point slices; DVE max/max_index extract per-slice top-8
    candidates (value + local index); a match_replace/compare/copy_predicated
    trick extracts the exact top-16 global indices per query with no per-row
    gather.
  Phase A: PE precomputes per-point neighbor-side MLP terms
      A_j = feat_j @ W1a + pos_j @ W1c            (fp16, stored to DRAM)
      B_i = feat_i @ W1b - pos_i @ W1c + b1       (fp16, kept in SBUF)
    since edge(i,j) @ W1 + b1 == A_j + B_i  (rel_pos decomposes linearly).
  Phase G: dma_gather (transpose mode) fetches A rows for all edges of one
    neighbor-rank k at a time -> G_k [128H, NQ]; DVE adds B, ACT applies
    tanh-gelu; PE accumulates sum_k gelu(A+B) @ (W2/16) into PSUM (the mean
    over 16 neighbors is absorbed into W2); biases are applied with ACT.
  Phase M: out = gelu(agg@W3+b3)@W4+b4 entirely on PE/ACT, column-major.
Output is written as [B, 64, NQ] in a fixed column permutation; the host
inverts the permutation and concatenates the 8 cores.
"""

import math
import numpy as np

B = 2
CIN = 32
H = 128
COUT = 64
KK = 16  # knn_k
FP = CIN + 3 + 1  # featT rows + posT rows + ones row = 36
KR = 36  # distance-score contraction rows (bf16-split exact products)


class Cfg:
    def __init__(self, N=16384, NQ=2048, SW=1024):
        self.N = N          # total points per batch
        self.NQ = NQ        # queries per core per batch
        self.SW = SW        # KNN slice width (candidate window per max call)
        self.NSL = N // SW  # slices
        self.NQB = NQ // 128
        self.CH = min(512, SW)      # psum matmul chunk
        self.QCH = min(512, NQ)     # query-dim matmul chunk
        self.NCAND = self.NSL * 8   # candidates per query
        self.PSH = int(math.log2(N // 128))  # A-row permutation shift
        self.ACH = min(2048, N)     # fp streaming chunk for phase A
        assert N % SW == 0 and SW % self.CH == 0 and NQ % 128 == 0
        assert self.NCAND >= 16 and NQ % 16 == 0


def build_nc(cfg: Cfg):
    import concourse.mybir as mybir
    from concourse.bacc import Bacc
    from concourse.tile import TileContext

    f32 = mybir.dt.float32
    f16 = mybir.dt.float16
    i16 = mybir.dt.int16
    u32 = mybir.dt.uint32
    AF = mybir.ActivationFunctionType
    ALU = mybir.AluOpType

    N, NQ, SW, NSL, NQB, CH = cfg.N, cfg.NQ, cfg.SW, cfg.NSL, cfg.NQB, cfg.CH
    NCAND, PSH, ACH, QCH = cfg.NCAND, cfg.PSH, cfg.ACH, cfg.QCH
    NQ16 = NQ // 16

    nc = Bacc("TRN2")

    bf16 = mybir.dt.bfloat16

    # ---- external inputs ----
    pkeyT = nc.dram_tensor("pkeyT", [B, KR, N], bf16, kind="ExternalInput")
    qkeyT = nc.dram_tensor("qkeyT", [B, KR, NQ], bf16, kind="ExternalInput")
    fpT1 = nc.dram_tensor("fpT1", [B, FP, N], f32, kind="ExternalInput")
    qfpT1 = nc.dram_tensor("qfpT1", [B, FP, NQ], f32, kind="ExternalInput")
    LA = nc.dram_tensor("LA", [FP, H], f32, kind="ExternalInput")
    LB = nc.dram_tensor("LB", [FP, H], f32, kind="ExternalInput")
    W2s = nc.dram_tensor("W2s", [H, COUT], f32, kind="ExternalInput")
    W3 = nc.dram_tensor("W3", [COUT, H], f32, kind="ExternalInput")
    W4 = nc.dram_tensor("W4", [H, COUT], f32, kind="ExternalInput")
    b2c = nc.dram_tensor("b2c", [COUT, 1], f32, kind="ExternalInput")
    b3c = nc.dram_tensor("b3c", [H, 1], f32, kind="ExternalInput")
    b4c = nc.dram_tensor("b4c", [COUT, 1], f32, kind="ExternalInput")
    ident = nc.dram_tensor("ident", [128, 128], f32, kind="ExternalInput")

    out_t = nc.dram_tensor("out", [B, COUT, NQ], f32, kind="ExternalOutput")

    # ---- internal DRAM scratch ----
    A16 = [nc.dram_tensor(f"A16_{b}", [N, H], f16, kind="Internal") for b in range(B)]
    # wrapped gather-index storage, flat [k, qb*128+qp]
    Wid = [nc.dram_tensor(f"Wid_{b}", [KK, NQ], i16, kind="Internal") for b in range(B)]

    with TileContext(nc) as tc:
        with (
            tc.tile_pool(name="pmm", bufs=2, space="PSUM") as pmm,      # [128,SW]
            tc.tile_pool(name="pagg", bufs=1, space="PSUM") as pagg,    # [64,NQ]
            tc.tile_pool(name="consts", bufs=1) as consts,
            tc.tile_pool(name="kslice", bufs=2) as kslice,
            tc.tile_pool(name="qk", bufs=2) as qkpool,
            tc.tile_pool(name="vi", bufs=2) as vipool,
            tc.tile_pool(name="fin", bufs=2) as finpool,
            tc.tile_pool(name="apool", bufs=2) as apool,
            tc.tile_pool(name="gpool", bufs=2) as gpool,
            tc.tile_pool(name="mpool", bufs=2) as mpool,
        ):
            # ---------- constants ----------
            LA_sb = consts.tile([FP, H], f32, tag="LA")
            nc.sync.dma_start(out=LA_sb, in_=LA[:, :])
            LB_sb = consts.tile([FP, H], f32, tag="LB")
            nc.sync.dma_start(out=LB_sb, in_=LB[:, :])
            # fp16 weights for the fp16 matmul stages (cast during DMA: SWDGE)
            W2_sb = consts.tile([H, COUT], f16, tag="W2")
            nc.gpsimd.dma_start(out=W2_sb, in_=W2s[:, :])
            W3_sb = consts.tile([COUT, H], f16, tag="W3")
            nc.gpsimd.dma_start(out=W3_sb, in_=W3[:, :])
            W4_sb = consts.tile([H, COUT], f16, tag="W4")
            nc.gpsimd.dma_start(out=W4_sb, in_=W4[:, :])
            b2_sb = consts.tile([COUT, 1], f32, tag="b2")
            nc.sync.dma_start(out=b2_sb, in_=b2c[:, :])
            b3_sb = consts.tile([H, 1], f32, tag="b3")
            nc.sync.dma_start(out=b3_sb, in_=b3c[:, :])
            b4_sb = consts.tile([COUT, 1], f32, tag="b4")
            nc.sync.dma_start(out=b4_sb, in_=b4c[:, :])
            id_sb = consts.tile([128, 128], f32, tag="ident")
            nc.sync.dma_start(out=id_sb, in_=ident[:, :])

            # slice-offset constants for candidate index reconstruction
            offs_sb = consts.tile([128, NCAND], f32, tag="offs")
            for sl in range(NSL):
                nc.vector.memset(offs_sb[:, sl * 8 : (sl + 1) * 8], float(sl * SW))
            neg1_sb = consts.tile([128, NCAND], f32, tag="neg1")
            nc.vector.memset(neg1_sb, -1.0)

            for b in range(B):
                # =================== Phase K: KNN ===================
                qk_sb = qkpool.tile([KR, NQ], bf16, tag="qk")
                nc.sync.dma_start(out=qk_sb, in_=qkeyT[b, :, :])

                # candidate values / local indices for all query blocks
                V_sb = vipool.tile([128, NQB * NCAND], f32, tag="V")
                I_sb = vipool.tile([128, NQB * NCAND], u32, tag="I")

                for sl in range(NSL):
                    pk_sb = kslice.tile([KR, SW], bf16, tag="pk")
                    nc.sync.dma_start(
                        out=pk_sb, in_=pkeyT[b, :, sl * SW : (sl + 1) * SW]
                    )
                    for qb in range(NQB):
                        ps = pmm.tile([128, SW], f32, tag="knn")
                        for c in range(SW // CH):
                            nc.tensor.matmul(
                                ps[:, c * CH : (c + 1) * CH],
                                lhsT=qk_sb[:, qb * 128 : (qb + 1) * 128],
                                rhs=pk_sb[:, c * CH : (c + 1) * CH],
                                start=True,
                                stop=True,
                            )
                        vslot = V_sb[:, qb * NCAND + sl * 8 : qb * NCAND + sl * 8 + 8]
                        islot = I_sb[:, qb * NCAND + sl * 8 : qb * NCAND + sl * 8 + 8]
                        nc.vector.max(out=vslot, in_=ps[:, :])
                        nc.vector.max_index(out=islot, in_max=vslot, in_values=ps[:, :])

                # final exact top-16 per query (no per-row gather):
                # match_replace marks the top-8 slots; a predicated copy + max
                # pulls out their (global) indices.
                IDXf = finpool.tile([128, NQB * KK], f32, tag="IDXf")
                for qb in range(NQB):
                    Vq = V_sb[:, qb * NCAND : (qb + 1) * NCAND]
                    Iq = I_sb[:, qb * NCAND : (qb + 1) * NCAND]
                    ci = finpool.tile([128, NCAND], f32, tag="ci")
                    nc.vector.tensor_copy(ci, Iq)  # u32 -> f32
                    nc.vector.tensor_add(ci, ci, offs_sb)
                    V2 = finpool.tile([128, NCAND], f32, tag="V2")
                    V3 = finpool.tile([128, NCAND], f32, tag="V3")
                    msk = finpool.tile([128, NCAND], mybir.dt.uint8, tag="msk")
                    sel = finpool.tile([128, NCAND], f32, tag="sel")
                    m8 = finpool.tile([128, 8], f32, tag="m8")
                    # round 1: top 8
                    nc.vector.max(out=m8, in_=Vq)
                    nc.vector.match_replace(
                        out=V2, in_to_replace=m8, in_values=Vq, imm_value=-1e30
                    )
                    nc.vector.tensor_tensor(msk, Vq, V2, ALU.not_equal)
                    nc.vector.tensor_copy(sel, neg1_sb)
                    nc.vector.copy_predicated(sel, msk, ci)
                    nc.vector.max(out=IDXf[:, qb * KK : qb * KK + 8], in_=sel)
                    # round 2: next 8
                    m8b = finpool.tile([128, 8], f32, tag="m8b")
                    nc.vector.max(out=m8b, in_=V2)
                    nc.vector.match_replace(
                        out=V3, in_to_replace=m8b, in_values=V2, imm_value=-1e30
                    )
                    nc.vector.tensor_tensor(msk, V2, V3, ALU.not_equal)
                    nc.vector.tensor_copy(sel, neg1_sb)
                    nc.vector.copy_predicated(sel, msk, ci)
                    nc.vector.max(out=IDXf[:, qb * KK + 8 : qb * KK + 16], in_=sel)

                # ---- index postprocess: permute for A16 layout, to wrapped DRAM ----
                # jperm = (j % 128) << PSH | (j >> 7)
                ji = finpool.tile([128, NQB * KK], u32, tag="ji")
                nc.vector.tensor_copy(ji, IDXf)  # f32 -> u32
                jlo = finpool.tile([128, NQB * KK], u32, tag="jlo")
                nc.vector.tensor_scalar(
                    jlo, ji, 127, scalar2=PSH, op0=ALU.bitwise_and,
                    op1=ALU.logical_shift_left,
                )
                nc.vector.tensor_scalar(
                    ji, ji, 7, scalar2=None, op0=ALU.logical_shift_right
                )
                nc.vector.tensor_tensor(ji, ji, jlo, ALU.bitwise_or)
                jf = finpool.tile([128, NQB * KK], f32, tag="jf")
                nc.vector.tensor_copy(jf, ji)  # u32 -> f32 (exact, < 2^14)

                # transpose [qp, (qb k)] -> [(qb k), qp] in chunks of 128 cols
                n_tr = (NQB * KK + 127) // 128
                for h in range(n_tr):
                    c0 = h * 128
                    cn = min(128, NQB * KK - c0)
                    pst = pmm.tile([128, 128], f32, tag="knn")
                    nc.tensor.transpose(pst[:cn, :], jf[:, c0 : c0 + cn], id_sb)
                    t16 = finpool.tile([128, 128], i16, tag="t16")
                    nc.vector.tensor_copy(t16[:cn, :], pst[:cn, :])  # f32->i16
                    qb0 = h * (128 // KK)
                    qbn = cn // KK
                    for j in range(qbn):
                        nc.sync.dma_start(
                            out=Wid[b][:, (qb0 + j) * 128 : (qb0 + j + 1) * 128],
                            in_=t16[j * KK : (j + 1) * KK, :],
                        )

                # =================== Phase A: A16 + BT ===================
                for t in range(N // ACH):
                    fp_sb = apool.tile([FP, ACH], f32, tag="fp")
                    nc.sync.dma_start(
                        out=fp_sb, in_=fpT1[b, :, t * ACH : (t + 1) * ACH]
                    )
                    for c4 in range(ACH // 512):
                        psA = pmm.tile([128, 512], f32, tag="knn")
                        for c in range(4):
                            j0 = c4 * 512 + c * 128
                            nc.tensor.matmul(
                                psA[:, c * 128 : (c + 1) * 128],
                                lhsT=fp_sb[:, j0 : j0 + 128],
                                rhs=LA_sb,
                                start=True,
                                stop=True,
                            )
                        a_sb = apool.tile([128, 512], f16, tag="a16")
                        nc.scalar.activation(a_sb, psA[:, :], AF.Copy)
                        # DRAM rows jperm = p*(N/128) + chunkidx
                        ci0 = (t * ACH) // 128 + c4 * 4
                        nc.sync.dma_start(
                            out=A16[b][:, :]
                            .rearrange("(p c) h -> p c h", p=128)[:, ci0 : ci0 + 4, :],
                            in_=a_sb.rearrange("p (c h) -> p c h", c=4),
                        )

                # BT in gather-column order: column m <-> query u(m)=(m%16)*NQ16+m//16
                BT_sb = gpool.tile([H, NQ], f16, tag="BT")
                qfp_sb = apool.tile([FP, NQ], f32, tag="qfp")
                nc.sync.dma_start(out=qfp_sb, in_=qfpT1[b, :, :])
                qfp_m = qfp_sb.rearrange("c (p16 col) -> c col p16", p16=16)
                qc16 = QCH // 16
                for i in range(NQ // QCH):
                    psB = pmm.tile([128, QCH], f32, tag="knn")
                    nc.tensor.matmul(
                        psB[:, :],
                        lhsT=LB_sb,
                        rhs=qfp_m[:, i * qc16 : (i + 1) * qc16, :],
                        start=True,
                        stop=True,
                    )
                    nc.scalar.activation(
                        BT_sb[:, i * QCH : (i + 1) * QCH], psB[:, :], AF.Copy
                    )

                # =================== Phase G: gather + edge MLP ===================
                idxw = gpool.tile([128, KK * NQ16], i16, tag="idxw")
                for r in range(8):  # replicate across the 8 gpsimd cores
                    nc.sync.dma_start(
                        out=idxw[r * 16 : (r + 1) * 16].rearrange(
                            "p (k col) -> p k col", k=KK
                        ),
                        in_=Wid[b][:, :].rearrange(
                            "k (p16 col) -> p16 k col", p16=16
                        ),
                    )
                ps_agg = pagg.tile([COUT, NQ], f32, tag="agg")
                for k in range(KK):
                    gk = gpool.tile([H, NQ], f16, tag="gk")
                    nc.gpsimd.dma_gather(
                        out_ap=gk.rearrange("p (one m) -> p one m", one=1),
                        in_ap=A16[b][:, :],
                        idxs_ap=idxw[:, k * NQ16 : (k + 1) * NQ16],
                        num_idxs=NQ,
                        num_idxs_reg=NQ,
                        elem_size=H,
                        transpose=True,
                        single_packet=False,
                    )
                    nc.vector.tensor_add(gk, gk, BT_sb)
                    g1 = gpool.tile([H, NQ], f16, tag="g1")
                    nc.scalar.activation(g1, gk, AF.Gelu_apprx_tanh)
                    for c in range(NQ // QCH):
                        nc.tensor.matmul(
                            ps_agg[:, c * QCH : (c + 1) * QCH],
                            lhsT=W2_sb,
                            rhs=g1[:, c * QCH : (c + 1) * QCH],
                            start=(k == 0),
                            stop=(k == KK - 1),
                        )

                # =================== Phase M: output MLP ===================
                agg_sb = mpool.tile([COUT, NQ], f16, tag="agg_sb")
                nc.scalar.activation(agg_sb, ps_agg[:, :], AF.Identity, bias=b2_sb)
                g2_sb = mpool.tile([H, NQ], f16, tag="g2")
                for c in range(NQ // QCH):
                    ps2 = pmm.tile([128, QCH], f32, tag="knn")
                    nc.tensor.matmul(
                        ps2[:, :],
                        lhsT=W3_sb,
                        rhs=agg_sb[:, c * QCH : (c + 1) * QCH],
                        start=True,
                        stop=True,
                    )
                    nc.scalar.activation(
                        g2_sb[:, c * QCH : (c + 1) * QCH],
                        ps2[:, :],
                        AF.Gelu_apprx_tanh,
                        bias=b3_sb,
                    )
                ps3 = pagg.tile([COUT, NQ], f32, tag="agg")
                for c in range(NQ // QCH):
                    nc.tensor.matmul(
                        ps3[:, c * QCH : (c + 1) * QCH],
                        lhsT=W4_sb,
                        rhs=g2_sb[:, c * QCH : (c + 1) * QCH],
                        start=True,
                        stop=True,
                    )
                o_sb = mpool.tile([COUT, NQ], f32, tag="osb")
                nc.scalar.activation(o_sb, ps3[:, :], AF.Identity, bias=b4_sb)
                nc.sync.dma_start(out=out_t[b, :, :], in_=o_sb)

    nc.finalize()
    return nc


def _split3(x):
    """3-term bf16 split of fp64 array: sum(h1,h2,h3) ~= x to ~2^-27."""
    import ml_dtypes

    bf = ml_dtypes.bfloat16
    h1 = x.astype(bf).astype(np.float64)
    r = x - h1
    h2 = r.astype(bf).astype(np.float64)
    h3 = (r - h2).astype(bf).astype(np.float64)
    return h1, h2, h3


def _build_knn_keys(posT):
    """Exact-product bf16 score rows: s[i,j] = -|q_i - p_j|^2 (+O(1e-8)).

    Returns (qrows, prows): [B, 36, M] bf16 each. All products of paired rows
    are exact in fp32 (bf16 x bf16), so PSUM accumulation is the only
    rounding; coords are centered so partial sums stay small.
    """
    import ml_dtypes

    bf = ml_dtypes.bfloat16
    Bn, _, M = posT.shape
    pc = posT.astype(np.float64) - 0.5
    qrows = np.zeros((Bn, KR, M), np.float64)
    prows = np.zeros((Bn, KR, M), np.float64)
    for c in range(3):
        u, v, w = _split3(pc[:, c])
        a1, a2, a3 = _split3(pc[:, c] * pc[:, c])
        r0 = c * 12
        # (q-row, p-row) pairs; partial sums stay bounded per coord
        qrows[:, r0 + 0], prows[:, r0 + 0] = 2 * u, u
        qrows[:, r0 + 1], prows[:, r0 + 1] = -a1, 1.0
        qrows[:, r0 + 2], prows[:, r0 + 2] = -1.0, a1
        qrows[:, r0 + 3], prows[:, r0 + 3] = 2 * u, v
        qrows[:, r0 + 4], prows[:, r0 + 4] = 2 * v, u
        qrows[:, r0 + 5], prows[:, r0 + 5] = -a2, 1.0
        qrows[:, r0 + 6], prows[:, r0 + 6] = -1.0, a2
        qrows[:, r0 + 7], prows[:, r0 + 7] = 2 * u, w
        qrows[:, r0 + 8], prows[:, r0 + 8] = 2 * w, u
        qrows[:, r0 + 9], prows[:, r0 + 9] = 2 * v, v
        qrows[:, r0 + 10], prows[:, r0 + 10] = -a3, 1.0
        qrows[:, r0 + 11], prows[:, r0 + 11] = -1.0, a3
    return qrows.astype(bf), prows.astype(bf)


def prep_inputs(positions, features, W1, b1, W2, b2, W3, b3, W4, b4, cfg: Cfg,
                n_cores=8):
    """Host-side input massaging: transposes/scalings/dtype splits only."""
    N, NQ = cfg.N, cfg.NQ
    pos = np.asarray(positions, np.float32)
    feat = np.asarray(features, np.float32)
    posT = pos.transpose(0, 2, 1)  # [B,3,N]
    qkeyT_full, pkeyT = _build_knn_keys(posT)
    fpT1 = np.concatenate(
        [feat.transpose(0, 2, 1), posT, np.ones((B, 1, N), np.float32)], 1
    ).astype(np.float32)  # [B,36,N]

    W1 = np.asarray(W1, np.float32)
    LA = np.concatenate([W1[:CIN], W1[2 * CIN :], np.zeros((1, H), np.float32)], 0)
    LB = np.concatenate(
        [W1[CIN : 2 * CIN], -W1[2 * CIN :], np.asarray(b1, np.float32)[None, :]], 0
    )
    shared = dict(
        pkeyT=pkeyT,
        fpT1=fpT1,
        LA=LA.astype(np.float32),
        LB=LB.astype(np.float32),
        W2s=(np.asarray(W2, np.float32) / float(KK)),
        W3=np.asarray(W3, np.float32),
        W4=np.asarray(W4, np.float32),
        b2c=np.asarray(b2, np.float32)[:, None],
        b3c=np.asarray(b3, np.float32)[:, None],
        b4c=np.asarray(b4, np.float32)[:, None],
        ident=np.eye(128, dtype=np.float32),
    )
    in_maps = []
    for c in range(n_cores):
        sl = slice(c * NQ, (c + 1) * NQ)
        m = dict(shared)
        m["qkeyT"] = np.ascontiguousarray(qkeyT_full[:, :, sl])
        m["qfpT1"] = np.ascontiguousarray(fpT1[:, :, sl])
        in_maps.append(m)
    return in_maps


def assemble_output(results, cfg: Cfg, n_cores=8):
    NQ = cfg.NQ
    NQ16 = NQ // 16
    m = np.arange(NQ)
    u = (m % 16) * NQ16 + m // 16  # column m holds query u(m)
    out = np.empty((B, n_cores * NQ, COUT), np.float32)
    for c in range(n_cores):
        o = results[c]["out"]  # [B, COUT, NQ]
        for b in range(B):
            out[b, c * NQ + u, :] = np.asarray(o[b]).T
    return out


_CACHED = {}


def _get_nc(cfg: Cfg):
    key = (cfg.N, cfg.NQ, cfg.SW)
    if key not in _CACHED:
        _CACHED[key] = build_nc(cfg)
    return _CACHED[key]


def kernel(positions, features, W1, b1, W2, b2, W3, b3, W4, b4, knn_k):
    from concourse.bass_utils import run_bass_kernel_spmd

    assert int(knn_k) == KK
    cfg = Cfg()
    nc = _get_nc(cfg)
    in_maps = prep_inputs(
        positions, features, W1, b1, W2, b2, W3, b3, W4, b4, cfg, n_cores=8
    )
    res = run_bass_kernel_spmd(nc, in_maps, core_ids=list(range(8)))
    return assemble_output(res.results, cfg, n_cores=8)


# revision 20
# speedup vs baseline: 1867.0710x; 1867.0710x over previous
"""Trainium2 Bass kernel for PointFeatureConv (KNN + edge MLP + mean + out MLP).

Self-contained: builds a Bass/Tile program, shards queries across 8 NeuronCores,
runs via run_bass_kernel_spmd, reassembles the full output.

Algorithm per core (data-parallel over queries; points replicated):
  Phase K (KNN): for each 128-query block, PE computes scores
      s[i,j] = 2*q_i.p_j - |p_j|^2   (argmax_j s = nearest neighbor)
    into PSUM in SW-point slices; DVE max/max_index extract per-slice top-8
    candidates (value + local index); a match_replace/compare/copy_predicated
    trick extracts the exact top-16 global indices per query with no per-row
    gather.
  Phase A: PE precomputes per-point neighbor-side MLP terms
      A_j = feat_j @ W1a + pos_j @ W1c            (fp16, stored to DRAM)
      B_i = feat_i @ W1b - pos_i @ W1c + b1       (fp16, kept in SBUF)
    since edge(i,j) @ W1 + b1 == A_j + B_i  (rel_pos decomposes linearly).
  Phase G: dma_gather (transpose mode) fetches A rows for all edges of one
    neighbor-rank k at a time -> G_k [128H, NQ]; DVE adds B, ACT applies
    tanh-gelu; PE accumulates sum_k gelu(A+B) @ (W2/16) into PSUM (the mean
    over 16 neighbors is absorbed into W2); biases are applied with ACT.
  Phase M: out = gelu(agg@W3+b3)@W4+b4 entirely on PE/ACT, column-major.
Output is written as [B, 64, NQ] in a fixed column permutation; the host
inverts the permutation and concatenates the 8 cores.
"""

import math
import numpy as np

B = 2
CIN = 32
H = 128
COUT = 64
KK = 16  # knn_k
FP = CIN + 3 + 1  # featT rows + posT rows + ones row = 36
KR = 36  # distance-score contraction rows (bf16-split exact products)


class Cfg:
    def __init__(self, N=16384, NQ=2048, SW=2048):
        self.N = N          # total points per batch
        self.NQ = NQ        # queries per core per batch
        self.SW = SW        # KNN slice width (candidate window per max call)
        self.NSL = N // SW  # slices
        self.NQB = NQ // 128
        self.CH = min(512, SW)      # psum matmul chunk
        self.QCH = min(512, NQ)     # query-dim matmul chunk
        self.NCAND = self.NSL * 8   # candidates per query
        self.PSH = int(math.log2(N // 128))  # A-row permutation shift
        self.ACH = min(2048, N)     # fp streaming chunk for phase A
        assert N % SW == 0 and SW % self.CH == 0 and NQ % 128 == 0
        assert self.NCAND >= 16 and NQ % 16 == 0


def build_nc(cfg: Cfg):
    import concourse.mybir as mybir
    from concourse.bacc import Bacc
    from concourse.tile import TileContext

    f32 = mybir.dt.float32
    f16 = mybir.dt.float16
    i16 = mybir.dt.int16
    u32 = mybir.dt.uint32
    AF = mybir.ActivationFunctionType
    ALU = mybir.AluOpType

    N, NQ, SW, NSL, NQB, CH = cfg.N, cfg.NQ, cfg.SW, cfg.NSL, cfg.NQB, cfg.CH
    NCAND, PSH, ACH, QCH = cfg.NCAND, cfg.PSH, cfg.ACH, cfg.QCH
    NQ16 = NQ // 16

    nc = Bacc("TRN2")

    bf16 = mybir.dt.bfloat16

    # ---- external inputs ----
    pkeyT = nc.dram_tensor("pkeyT", [B, KR, N], bf16, kind="ExternalInput")
    qkeyT = nc.dram_tensor("qkeyT", [B, KR, NQ], bf16, kind="ExternalInput")
    fpT1 = nc.dram_tensor("fpT1", [B, FP, N], f32, kind="ExternalInput")
    qfpT1 = nc.dram_tensor("qfpT1", [B, FP, NQ], f32, kind="ExternalInput")
    LA = nc.dram_tensor("LA", [FP, H], f32, kind="ExternalInput")
    LB = nc.dram_tensor("LB", [FP, H], f32, kind="ExternalInput")
    W2s = nc.dram_tensor("W2s", [H, COUT], f32, kind="ExternalInput")
    W3 = nc.dram_tensor("W3", [COUT, H], f32, kind="ExternalInput")
    W4 = nc.dram_tensor("W4", [H, COUT], f32, kind="ExternalInput")
    b2c = nc.dram_tensor("b2c", [COUT, 1], f32, kind="ExternalInput")
    b3c = nc.dram_tensor("b3c", [H, 1], f32, kind="ExternalInput")
    b4c = nc.dram_tensor("b4c", [COUT, 1], f32, kind="ExternalInput")
    ident = nc.dram_tensor("ident", [128, 128], f32, kind="ExternalInput")

    out_t = nc.dram_tensor("out", [B, COUT, NQ], f32, kind="ExternalOutput")

    # ---- internal DRAM scratch ----
    A16 = [nc.dram_tensor(f"A16_{b}", [N, H], f16, kind="Internal") for b in range(B)]
    # wrapped gather-index storage, flat [k, qb*128+qp]
    Wid = [nc.dram_tensor(f"Wid_{b}", [KK, NQ], i16, kind="Internal") for b in range(B)]

    with TileContext(nc) as tc:
        with (
            tc.tile_pool(name="pmm", bufs=2, space="PSUM") as pmm,      # [128,SW]
            tc.tile_pool(name="consts", bufs=1) as consts,
            tc.tile_pool(name="kslice", bufs=2) as kslice,
            tc.tile_pool(name="qk", bufs=2) as qkpool,
            tc.tile_pool(name="vi", bufs=2) as vipool,
            tc.tile_pool(name="fin", bufs=2) as finpool,
            tc.tile_pool(name="apool", bufs=2) as apool,
            tc.tile_pool(name="gpool", bufs=2) as gpool,
            tc.tile_pool(name="mpool", bufs=2) as mpool,
        ):
            # ---------- constants ----------
            LA_sb = consts.tile([FP, H], f32, tag="LA")
            nc.sync.dma_start(out=LA_sb, in_=LA[:, :])
            LB_sb = consts.tile([FP, H], f32, tag="LB")
            nc.sync.dma_start(out=LB_sb, in_=LB[:, :])
            # fp16 weights for the fp16 matmul stages (cast during DMA: SWDGE)
            W2_sb = consts.tile([H, COUT], f16, tag="W2")
            nc.gpsimd.dma_start(out=W2_sb, in_=W2s[:, :])
            W3_sb = consts.tile([COUT, H], f16, tag="W3")
            nc.gpsimd.dma_start(out=W3_sb, in_=W3[:, :])
            W4_sb = consts.tile([H, COUT], f16, tag="W4")
            nc.gpsimd.dma_start(out=W4_sb, in_=W4[:, :])
            b2_sb = consts.tile([COUT, 1], f32, tag="b2")
            nc.sync.dma_start(out=b2_sb, in_=b2c[:, :])
            b3_sb = consts.tile([H, 1], f32, tag="b3")
            nc.sync.dma_start(out=b3_sb, in_=b3c[:, :])
            b4_sb = consts.tile([COUT, 1], f32, tag="b4")
            nc.sync.dma_start(out=b4_sb, in_=b4c[:, :])
            id_sb = consts.tile([128, 128], f32, tag="ident")
            nc.sync.dma_start(out=id_sb, in_=ident[:, :])

            # slice-offset constants for candidate index reconstruction
            offs_sb = consts.tile([128, NCAND], f32, tag="offs")
            for sl in range(NSL):
                nc.vector.memset(offs_sb[:, sl * 8 : (sl + 1) * 8], float(sl * SW))
            neg1_sb = consts.tile([128, NCAND], f32, tag="neg1")
            nc.vector.memset(neg1_sb, -1.0)

            for b in range(B):
                # =================== Phase K: KNN ===================
                qk_sb = qkpool.tile([KR, NQ], bf16, tag="qk")
                nc.sync.dma_start(out=qk_sb, in_=qkeyT[b, :, :])

                # candidate values / local indices for all query blocks
                V_sb = vipool.tile([128, NQB * NCAND], f32, tag="V")
                I_sb = vipool.tile([128, NQB * NCAND], u32, tag="I")

                for sl in range(NSL):
                    pk_sb = kslice.tile([KR, SW], bf16, tag="pk")
                    nc.sync.dma_start(
                        out=pk_sb, in_=pkeyT[b, :, sl * SW : (sl + 1) * SW]
                    )
                    for qb in range(NQB):
                        ps = pmm.tile([128, SW], f32, tag="ps")
                        for c in range(SW // CH):
                            nc.tensor.matmul(
                                ps[:, c * CH : (c + 1) * CH],
                                lhsT=qk_sb[:, qb * 128 : (qb + 1) * 128],
                                rhs=pk_sb[:, c * CH : (c + 1) * CH],
                                start=True,
                                stop=True,
                            )
                        vslot = V_sb[:, qb * NCAND + sl * 8 : qb * NCAND + sl * 8 + 8]
                        islot = I_sb[:, qb * NCAND + sl * 8 : qb * NCAND + sl * 8 + 8]
                        nc.vector.max(out=vslot, in_=ps[:, :])
                        nc.vector.max_index(out=islot, in_max=vslot, in_values=ps[:, :])

                # final exact top-16 per query (no per-row gather):
                # match_replace marks the top-8 slots; a predicated copy + max
                # pulls out their (global) indices.
                IDXf = finpool.tile([128, NQB * KK], f32, tag="IDXf")
                for qb in range(NQB):
                    Vq = V_sb[:, qb * NCAND : (qb + 1) * NCAND]
                    Iq = I_sb[:, qb * NCAND : (qb + 1) * NCAND]
                    ci = finpool.tile([128, NCAND], f32, tag="ci")
                    nc.vector.tensor_copy(ci, Iq)  # u32 -> f32
                    nc.vector.tensor_add(ci, ci, offs_sb)
                    V2 = finpool.tile([128, NCAND], f32, tag="V2")
                    V3 = finpool.tile([128, NCAND], f32, tag="V3")
                    msk = finpool.tile([128, NCAND], mybir.dt.uint8, tag="msk")
                    sel = finpool.tile([128, NCAND], f32, tag="sel")
                    m8 = finpool.tile([128, 8], f32, tag="m8")
                    # round 1: top 8
                    nc.vector.max(out=m8, in_=Vq)
                    nc.vector.match_replace(
                        out=V2, in_to_replace=m8, in_values=Vq, imm_value=-1e30
                    )
                    nc.vector.tensor_tensor(msk, Vq, V2, ALU.not_equal)
                    nc.vector.tensor_copy(sel, neg1_sb)
                    nc.vector.copy_predicated(sel, msk, ci)
                    nc.vector.max(out=IDXf[:, qb * KK : qb * KK + 8], in_=sel)
                    # round 2: next 8
                    m8b = finpool.tile([128, 8], f32, tag="m8b")
                    nc.vector.max(out=m8b, in_=V2)
                    nc.vector.match_replace(
                        out=V3, in_to_replace=m8b, in_values=V2, imm_value=-1e30
                    )
                    nc.vector.tensor_tensor(msk, V2, V3, ALU.not_equal)
                    nc.vector.tensor_copy(sel, neg1_sb)
                    nc.vector.copy_predicated(sel, msk, ci)
                    nc.vector.max(out=IDXf[:, qb * KK + 8 : qb * KK + 16], in_=sel)

                # ---- index postprocess: permute for A16 layout, to wrapped DRAM ----
                # jperm = (j % 128) << PSH | (j >> 7)
                ji = finpool.tile([128, NQB * KK], u32, tag="ji")
                nc.vector.tensor_copy(ji, IDXf)  # f32 -> u32
                jlo = finpool.tile([128, NQB * KK], u32, tag="jlo")
                nc.vector.tensor_scalar(
                    jlo, ji, 127, scalar2=PSH, op0=ALU.bitwise_and,
                    op1=ALU.logical_shift_left,
                )
                nc.vector.tensor_scalar(
                    ji, ji, 7, scalar2=None, op0=ALU.logical_shift_right
                )
                nc.vector.tensor_tensor(ji, ji, jlo, ALU.bitwise_or)
                jf = finpool.tile([128, NQB * KK], f32, tag="jf")
                nc.vector.tensor_copy(jf, ji)  # u32 -> f32 (exact, < 2^14)

                # transpose [qp, (qb k)] -> [(qb k), qp] in chunks of 128 cols
                n_tr = (NQB * KK + 127) // 128
                for h in range(n_tr):
                    c0 = h * 128
                    cn = min(128, NQB * KK - c0)
                    pst = pmm.tile([128, 128], f32, tag="ps")
                    nc.tensor.transpose(pst[:cn, :], jf[:, c0 : c0 + cn], id_sb)
                    t16 = finpool.tile([128, 128], i16, tag="t16")
                    nc.vector.tensor_copy(t16[:cn, :], pst[:cn, :])  # f32->i16
                    qb0 = h * (128 // KK)
                    qbn = cn // KK
                    for j in range(qbn):
                        nc.sync.dma_start(
                            out=Wid[b][:, (qb0 + j) * 128 : (qb0 + j + 1) * 128],
                            in_=t16[j * KK : (j + 1) * KK, :],
                        )

                # =================== Phase A: A16 + BT ===================
                for t in range(N // ACH):
                    fp_sb = apool.tile([FP, ACH], f32, tag="fp")
                    nc.sync.dma_start(
                        out=fp_sb, in_=fpT1[b, :, t * ACH : (t + 1) * ACH]
                    )
                    for c4 in range(ACH // 512):
                        psA = pmm.tile([128, 512], f32, tag="ps")
                        for c in range(4):
                            j0 = c4 * 512 + c * 128
                            nc.tensor.matmul(
                                psA[:, c * 128 : (c + 1) * 128],
                                lhsT=fp_sb[:, j0 : j0 + 128],
                                rhs=LA_sb,
                                start=True,
                                stop=True,
                            )
                        a_sb = apool.tile([128, 512], f16, tag="a16")
                        nc.scalar.activation(a_sb, psA[:, :], AF.Copy)
                        # DRAM rows jperm = p*(N/128) + chunkidx
                        ci0 = (t * ACH) // 128 + c4 * 4
                        nc.sync.dma_start(
                            out=A16[b][:, :]
                            .rearrange("(p c) h -> p c h", p=128)[:, ci0 : ci0 + 4, :],
                            in_=a_sb.rearrange("p (c h) -> p c h", c=4),
                        )

                # BT in gather-column order: column m <-> query u(m)=(m%16)*NQ16+m//16
                BT_sb = gpool.tile([H, NQ], f16, tag="BT")
                qfp_sb = apool.tile([FP, NQ], f32, tag="qfp")
                nc.sync.dma_start(out=qfp_sb, in_=qfpT1[b, :, :])
                qfp_m = qfp_sb.rearrange("c (p16 col) -> c col p16", p16=16)
                qc16 = QCH // 16
                for i in range(NQ // QCH):
                    psB = pmm.tile([128, QCH], f32, tag="ps")
                    nc.tensor.matmul(
                        psB[:, :],
                        lhsT=LB_sb,
                        rhs=qfp_m[:, i * qc16 : (i + 1) * qc16, :],
                        start=True,
                        stop=True,
                    )
                    nc.scalar.activation(
                        BT_sb[:, i * QCH : (i + 1) * QCH], psB[:, :], AF.Copy
                    )

                # =================== Phase G: gather + edge MLP ===================
                idxw = gpool.tile([128, KK * NQ16], i16, tag="idxw")
                for r in range(8):  # replicate across the 8 gpsimd cores
                    nc.sync.dma_start(
                        out=idxw[r * 16 : (r + 1) * 16].rearrange(
                            "p (k col) -> p k col", k=KK
                        ),
                        in_=Wid[b][:, :].rearrange(
                            "k (p16 col) -> p16 k col", p16=16
                        ),
                    )
                ps_agg = pmm.tile([COUT, NQ], f32, tag="ps")
                for k in range(KK):
                    gk = gpool.tile([H, NQ], f16, tag="gk")
                    nc.gpsimd.dma_gather(
                        out_ap=gk.rearrange("p (one m) -> p one m", one=1),
                        in_ap=A16[b][:, :],
                        idxs_ap=idxw[:, k * NQ16 : (k + 1) * NQ16],
                        num_idxs=NQ,
                        num_idxs_reg=NQ,
                        elem_size=H,
                        transpose=True,
                        single_packet=False,
                    )
                    nc.vector.tensor_add(gk, gk, BT_sb)
                    g1 = gpool.tile([H, NQ], f16, tag="g1")
                    nc.scalar.activation(g1, gk, AF.Gelu_apprx_tanh)
                    for c in range(NQ // QCH):
                        nc.tensor.matmul(
                            ps_agg[:, c * QCH : (c + 1) * QCH],
                            lhsT=W2_sb,
                            rhs=g1[:, c * QCH : (c + 1) * QCH],
                            start=(k == 0),
                            stop=(k == KK - 1),
                        )

                # =================== Phase M: output MLP ===================
                agg_sb = mpool.tile([COUT, NQ], f16, tag="agg_sb")
                nc.scalar.activation(agg_sb, ps_agg[:, :], AF.Identity, bias=b2_sb)
                g2_sb = mpool.tile([H, NQ], f16, tag="g2")
                for c in range(NQ // QCH):
                    ps2 = pmm.tile([128, QCH], f32, tag="ps")
                    nc.tensor.matmul(
                        ps2[:, :],
                        lhsT=W3_sb,
                        rhs=agg_sb[:, c * QCH : (c + 1) * QCH],
                        start=True,
                        stop=True,
                    )
                    nc.scalar.activation(
                        g2_sb[:, c * QCH : (c + 1) * QCH],
                        ps2[:, :],
                        AF.Gelu_apprx_tanh,
                        bias=b3_sb,
                    )
                ps3 = pmm.tile([COUT, NQ], f32, tag="ps")
                for c in range(NQ // QCH):
                    nc.tensor.matmul(
                        ps3[:, c * QCH : (c + 1) * QCH],
                        lhsT=W4_sb,
                        rhs=g2_sb[:, c * QCH : (c + 1) * QCH],
                        start=True,
                        stop=True,
                    )
                o_sb = mpool.tile([COUT, NQ], f32, tag="osb")
                nc.scalar.activation(o_sb, ps3[:, :], AF.Identity, bias=b4_sb)
                nc.sync.dma_start(out=out_t[b, :, :], in_=o_sb)

    nc.finalize()
    return nc


def _split3(x):
    """3-term bf16 split of fp64 array: sum(h1,h2,h3) ~= x to ~2^-27."""
    import ml_dtypes

    bf = ml_dtypes.bfloat16
    h1 = x.astype(bf).astype(np.float64)
    r = x - h1
    h2 = r.astype(bf).astype(np.float64)
    h3 = (r - h2).astype(bf).astype(np.float64)
    return h1, h2, h3


def _build_knn_keys(posT):
    """Exact-product bf16 score rows: s[i,j] = -|q_i - p_j|^2 (+O(1e-8)).

    Returns (qrows, prows): [B, 36, M] bf16 each. All products of paired rows
    are exact in fp32 (bf16 x bf16), so PSUM accumulation is the only
    rounding; coords are centered so partial sums stay small.
    """
    import ml_dtypes

    bf = ml_dtypes.bfloat16
    Bn, _, M = posT.shape
    pc = posT.astype(np.float64) - 0.5
    qrows = np.zeros((Bn, KR, M), np.float64)
    prows = np.zeros((Bn, KR, M), np.float64)
    for c in range(3):
        u, v, w = _split3(pc[:, c])
        a1, a2, a3 = _split3(pc[:, c] * pc[:, c])
        r0 = c * 12
        # (q-row, p-row) pairs; partial sums stay bounded per coord
        qrows[:, r0 + 0], prows[:, r0 + 0] = 2 * u, u
        qrows[:, r0 + 1], prows[:, r0 + 1] = -a1, 1.0
        qrows[:, r0 + 2], prows[:, r0 + 2] = -1.0, a1
        qrows[:, r0 + 3], prows[:, r0 + 3] = 2 * u, v
        qrows[:, r0 + 4], prows[:, r0 + 4] = 2 * v, u
        qrows[:, r0 + 5], prows[:, r0 + 5] = -a2, 1.0
        qrows[:, r0 + 6], prows[:, r0 + 6] = -1.0, a2
        qrows[:, r0 + 7], prows[:, r0 + 7] = 2 * u, w
        qrows[:, r0 + 8], prows[:, r0 + 8] = 2 * w, u
        qrows[:, r0 + 9], prows[:, r0 + 9] = 2 * v, v
        qrows[:, r0 + 10], prows[:, r0 + 10] = -a3, 1.0
        qrows[:, r0 + 11], prows[:, r0 + 11] = -1.0, a3
    return qrows.astype(bf), prows.astype(bf)


def prep_inputs(positions, features, W1, b1, W2, b2, W3, b3, W4, b4, cfg: Cfg,
                n_cores=8):
    """Host-side input massaging: transposes/scalings/dtype splits only."""
    N, NQ = cfg.N, cfg.NQ
    pos = np.asarray(positions, np.float32)
    feat = np.asarray(features, np.float32)
    posT = pos.transpose(0, 2, 1)  # [B,3,N]
    qkeyT_full, pkeyT = _build_knn_keys(posT)
    fpT1 = np.concatenate(
        [feat.transpose(0, 2, 1), posT, np.ones((B, 1, N), np.float32)], 1
    ).astype(np.float32)  # [B,36,N]

    W1 = np.asarray(W1, np.float32)
    LA = np.concatenate([W1[:CIN], W1[2 * CIN :], np.zeros((1, H), np.float32)], 0)
    LB = np.concatenate(
        [W1[CIN : 2 * CIN], -W1[2 * CIN :], np.asarray(b1, np.float32)[None, :]], 0
    )
    shared = dict(
        pkeyT=pkeyT,
        fpT1=fpT1,
        LA=LA.astype(np.float32),
        LB=LB.astype(np.float32),
        W2s=(np.asarray(W2, np.float32) / float(KK)),
        W3=np.asarray(W3, np.float32),
        W4=np.asarray(W4, np.float32),
        b2c=np.asarray(b2, np.float32)[:, None],
        b3c=np.asarray(b3, np.float32)[:, None],
        b4c=np.asarray(b4, np.float32)[:, None],
        ident=np.eye(128, dtype=np.float32),
    )
    in_maps = []
    for c in range(n_cores):
        sl = slice(c * NQ, (c + 1) * NQ)
        m = dict(shared)
        m["qkeyT"] = np.ascontiguousarray(qkeyT_full[:, :, sl])
        m["qfpT1"] = np.ascontiguousarray(fpT1[:, :, sl])
        in_maps.append(m)
    return in_maps


def assemble_output(results, cfg: Cfg, n_cores=8):
    NQ = cfg.NQ
    NQ16 = NQ // 16
    m = np.arange(NQ)
    u = (m % 16) * NQ16 + m // 16  # column m holds query u(m)
    out = np.empty((B, n_cores * NQ, COUT), np.float32)
    for c in range(n_cores):
        o = results[c]["out"]  # [B, COUT, NQ]
        for b in range(B):
            out[b, c * NQ + u, :] = np.asarray(o[b]).T
    return out


_CACHED = {}


def _get_nc(cfg: Cfg):
    key = (cfg.N, cfg.NQ, cfg.SW)
    if key not in _CACHED:
        _CACHED[key] = build_nc(cfg)
    return _CACHED[key]


def kernel(positions, features, W1, b1, W2, b2, W3, b3, W4, b4, knn_k):
    from concourse.bass_utils import run_bass_kernel_spmd

    assert int(knn_k) == KK
    cfg = Cfg()
    nc = _get_nc(cfg)
    in_maps = prep_inputs(
        positions, features, W1, b1, W2, b2, W3, b3, W4, b4, cfg, n_cores=8
    )
    res = run_bass_kernel_spmd(nc, in_maps, core_ids=list(range(8)))
    return assemble_output(res.results, cfg, n_cores=8)
